# revision 32
# baseline (speedup 1.0000x reference)
"""Multi-head self-attention (B=4, S=2048, D=512, H=8, d=64) on 8 trn2 cores.

Sharding: 2 cores per batch element; each core computes 4 heads (a 256-wide
column slice of Wq/Wk/Wv and row slice of Wh) and produces a partial
[S, 512] output; the host sums the two partials per batch and adds
bh_eff = bh + bv @ Wh (the V-bias folded out of the device loop).

Per-core pipeline (measured ~153.5us/rep on HW, PE-work floor 139.9us):
  A) f32r projections of qT/kT (bias via ScalarE Identity) and of v,
     which is stored fp16, augmented with a ones column per head so the
     attention matmul also produces the softmax denominator row.
     (fp8 DoubleRow projections give 2x on these K=512 matmuls but the
     ~2% q/k element noise passes through softmax UNaveraged — attn is
     itself a random-sign average over values — costing 4e-2 rel err.)
  B) attention as ONE continuous 128-step software-pipelined stream over
     all 8 (head-pair, sq-quarter) windows; per step: the sc pair for
     step s (heads packed on PE row-groups, K=64 at base partitions
     0/64), exp of s, then the av matmuls — h0 rides 1 step behind its
     scores (ScalarE true Exp -> fp16, 612ns fits the 854ns t-period),
     h1 rides 2 steps behind (DVE linear-exp: i16(x*c1+c2) bitcast to
     fp16, a 2^x segment approximation, bias-centred, 3 of every 4 t;
     the extra step absorbs DVE's queued finisher jobs).  Window
     finishers (Z-row reciprocal, 1/Z ones-matmul broadcast via a
     borrowed sc slot, and the normalize multiplies into attnT) drain
     one-per-odd-t from a job queue, so no engine sees a burst.
  C) out[s,:] = attnT.T @ Wh as a post-stream phase; with qT/kT/v_aug
     double-buffered by rep parity it overlaps the next rep's
     projections, which is what the nrep-slope timing measures.

PSUM: 2x sc [128,1024] (4 banks, also lent to bc-broadcast and out-proj
tiles) + 4x av [128,512] (avs live across one window boundary for the
deferred normalize).  PE is the wall: scores 131k + attnV 131k cycles
are shape/port-bound at 128 elem/cycle (fp8 cannot help K<=128
matmuls), projections 49k, out 16k, bc 8k.  The exp split keeps ScalarE
(~98us) and DVE (~96us) under it.
"""

import numpy as np

NUM_HEADS = 8
D_MODEL = 512
D_HEAD = 64
B = 4
S = 2048
H_PER_CORE = 4          # heads per core
DQ = H_PER_CORE * D_HEAD  # 256 = per-core q/k/v width
N_CORES = 8
SCALE = 1.0 / np.sqrt(D_HEAD)

_KO = D_MODEL // 128    # 4 contraction chunks for the projections
_NT = S // 128          # 16 tiles of 128 along S
_VW = D_HEAD + 1        # 65: v columns per head incl. ones column

# DVE linear-exp: exp(SCALE*x) ~= bitcast_f16(i16(x*C1 + C2)).
# C2 includes -58.7 to centre the 2^frac-vs-1+frac sawtooth (mean bias
# e^0.0397), so DVE-half probs are not systematically ~4% above the
# ScalarE-half probs feeding the same softmax row.
_C1 = float(SCALE * np.log2(np.e) * 1024.0)
_C2 = float(15 * 1024 - 58.66)


def _split_excess_waits(nc):
    """Walrus's TRN2 codegen fits very few sync-waits per instruction (one on
    a Matmult's weight-load, few on drains).  Move excess waits onto NoOps
    inserted just before the instruction — engine queues are in-order, so a
    wait on a preceding same-engine instruction still protects it."""
    import concourse.mybir as mybir

    n_fixed = 0
    for f in nc.m.functions:
        for bb in f.blocks:
            insts = list(bb.instructions)
            out = []
            changed = False
            for ins in insts:
                si = ins.sync_info
                if si is not None and si.on_wait and len(si.on_wait) > 1:
                    waits = list(si.on_wait)
                    # An exp/matmul waiting on its OWN engine's completion sem
                    # is a slot-recycle WAW guard: implied by in-order issue,
                    # with the interleaved cross-engine reader guarded by the
                    # remaining wait.  Dropping it avoids a NoOp on the
                    # bottleneck queue (one per exp otherwise).
                    if isinstance(ins, (mybir.InstActivation, mybir.InstMatmult)):
                        eng_pfx = str(ins.engine).split(".")[-1] + "_"
                        cross = [w for w in waits
                                 if not str(getattr(w, "ant_name", "")).startswith(eng_pfx)]
                        if cross and len(cross) < len(waits):
                            waits = cross
                    for j, w in enumerate(waits[1:]):
                        nop = mybir.InstNoOp(
                            name=f"{ins.name}_waitnop{j}", ins=[], outs=[])
                        nop.engine = ins.engine
                        nop.sync_info = mybir.SyncInfo(on_wait=[w], on_update=[])
                        out.append(nop)
                    ins.sync_info = mybir.SyncInfo(
                        on_wait=waits[:1], on_update=list(si.on_update or []))
                    n_fixed += 1
                    changed = True
                out.append(ins)
            if changed:
                bb.instructions = out
    return n_fixed


def build_nc(nrep=1):
    """Build the per-core Bass program.  nrep>1 repeats the compute body
    (same tiles, idempotent) for wall-clock timing amplification."""
    import concourse.bass as bass
    import concourse.mybir as mybir
    import concourse.tile as tile

    f32 = mybir.dt.float32
    f32r = mybir.dt.float32r
    f16 = mybir.dt.float16
    i16 = mybir.dt.int16
    AF = mybir.ActivationFunctionType
    ALU = mybir.AluOpType

    nc = bass.Bass()
    x_d = nc.dram_tensor("x", [D_MODEL, S], f32r, kind="ExternalInput")
    wq_d = nc.dram_tensor("wq", [D_MODEL, DQ], f32r, kind="ExternalInput")
    wk_d = nc.dram_tensor("wk", [D_MODEL, DQ], f32r, kind="ExternalInput")
    wv_d = nc.dram_tensor("wv", [D_MODEL, DQ], f32r, kind="ExternalInput")
    wh_d = nc.dram_tensor("wh", [DQ, D_MODEL], f32r, kind="ExternalInput")
    bq_d = nc.dram_tensor("bq", [DQ], f32, kind="ExternalInput")
    bk_d = nc.dram_tensor("bk", [DQ], f32, kind="ExternalInput")
    o_d = nc.dram_tensor("o", [S, D_MODEL], f32, kind="ExternalOutput")

    with (
        nc.allow_low_precision(reason="f32r/fp16/fp8 attention pipeline"),
        tile.TileContext(nc) as tc,
        tc.tile_pool(name="cst", bufs=1) as cst,
        tc.tile_pool(name="big", bufs=1) as big,
        tc.tile_pool(name="pr", bufs=6) as pr,
        tc.tile_pool(name="ps_sc", bufs=2, space="PSUM") as ps_sc,
        tc.tile_pool(name="ps_av", bufs=4, space="PSUM") as ps_av,
    ):
        ones_col = cst.tile([1, D_HEAD], f32r)
        nc.gpsimd.memset(ones_col.bitcast(f32)[:], 1.0)

        # ---- phased input DMAs: what the first projections need gets the
        # full DMA-engine pool; the rest is gated behind a 1-elem copy that
        # depends on the first x chunk, so it cannot steal early bandwidth.
        w_shp = {"wq": (128, _KO, DQ), "wk": (128, _KO, DQ),
                 "wv": (128, _KO, DQ), "wh": (128, DQ // 128, D_MODEL)}
        w_dram = {"wq": wq_d, "wk": wk_d, "wv": wv_d, "wh": wh_d}
        w_sb = {}
        for n, s in w_shp.items():
            wt = big.tile(list(s), f32r, tag=f"w_{n}", name=f"w_{n}")
            w_sb[n] = wt
        xT = big.tile([128, _KO, S], f32r, tag="xT")
        x_rr = x_d.rearrange("(a p) s -> p a s", p=128)

        def load_x(sg, eng):
            eng.dma_start(xT[:, :, sg * 256:(sg + 1) * 256],
                          x_rr[:, :, sg * 256:(sg + 1) * 256])

        nc.sync.dma_start(w_sb["wk"][:],
                          w_dram["wk"].rearrange("(a p) m -> p a m", p=128))
        for sg, eng in ((0, nc.scalar), (1, nc.gpsimd), (2, nc.sync),
                        (3, nc.scalar)):
            load_x(sg, eng)
        bias_sb = {}
        for name, dram in (("bq", bq_d), ("bk", bk_d)):
            bt = cst.tile([128, DQ // 128], f32, tag=f"b_{name}")
            nc.gpsimd.dma_start(bt[:], dram.rearrange("(o p) -> p o", p=128))
            bias_sb[name] = bt
        # gate: a dependent 1-elem copy; phase-2 DMAs queue behind it
        gate = cst.tile([1, 1], f32)
        nc.gpsimd.tensor_copy(gate[:], xT.bitcast(f32)[0:1, 0, 0:1])
        for sg, eng in ((4, nc.gpsimd), (5, nc.gpsimd), (6, nc.gpsimd),
                        (7, nc.gpsimd)):
            load_x(sg, eng)
        for name in ("wq", "wv", "wh"):
            nc.gpsimd.dma_start(
                w_sb[name][:],
                w_dram[name].rearrange("(a p) m -> p a m", p=128))

        for _rep in range(nrep):
            # qT/kT/v_aug alternate buffers by rep parity so rep k+1's
            # projections never WAR-serialize against rep k's attention
            # (the nrep>1 timing build measures steady-state throughput;
            # with one buffer the proj/attention overlap is lost at every
            # rep boundary).  attnT needs no alternation: its first write
            # in rep k+1 happens ~a full window after rep k's last read.
            par = _rep % 2
            qT = big.tile([128, DQ // 128, S], f32r, tag=f"qT{par}",
                          name=f"qT{par}")
            kT = big.tile([128, DQ // 128, S], f32r, tag=f"kT{par}",
                          name=f"kT{par}")
            attnT = big.tile([128, DQ // 128, S], f32r, tag="attnT")
            v_aug = big.tile([128, _NT, H_PER_CORE * _VW], f16,
                             tag=f"v_aug{par}", name=f"v_aug{par}")
            nc.gpsimd.memset(v_aug[:], 1.0)

            def proj_group(dst, wname, bname, o, sg):
                p = ps_sc.tile([128, 1024], f32, tag="sc", name="p_qk")
                for ko in range(_KO):
                    nc.tensor.matmul(
                        p[:, :512],
                        w_sb[wname][:, ko, o * 128:(o + 1) * 128],
                        xT[:, ko, sg * 512:(sg + 1) * 512],
                        start=(ko == 0), stop=(ko == _KO - 1))
                nc.scalar.activation(
                    dst[:, o, sg * 512:(sg + 1) * 512], p[:, :512],
                    AF.Identity, bias=bias_sb[bname][:, o:o + 1])

            def project_v(t0, t1):
                for t in range(t0, t1):
                    pv = ps_av.tile([128, 512], f32, tag="av", name="pv")
                    for ko in range(_KO):
                        nc.tensor.matmul(
                            pv[:, :DQ],
                            xT[:, ko, t * 128:(t + 1) * 128],
                            w_sb["wv"][:, ko, :],
                            start=(ko == 0), stop=(ko == _KO - 1))
                    nc.vector.tensor_copy(
                        v_aug[:, t, :].rearrange(
                            "p (h w) -> p h w", w=_VW)[:, :, :D_HEAD],
                        pv[:, :DQ].rearrange("p (h w) -> p h w", w=D_HEAD))

            # ---- attention: one continuous 128-step stream over all 8
            # (pair, quarter) windows.  Per step: sc-pair for step s, exp(s),
            # av-pair for step s-1 — so each window's first scores fill the
            # previous window's exp-latency tail instead of stalling behind
            # its last av.  Window finishers (reciprocal+broadcast, the
            # normalize multiplies, and the out-projection sg-pieces) go into
            # a job queue drained one-per-odd-t, spreading DVE/PE extras
            # evenly between the exp slots.
            qps = [(o, jq) for jq in range(4) for o in (0, 1)]
            avs = {}
            jobs = []

            def emit_av_h(qi, t, p, which):
                o, jq = qps[qi]
                av = avs[qi][which]
                hh = 2 * o + which
                nc.tensor.matmul(
                    av[0:_VW, :],
                    v_aug[:, t, hh * _VW:(hh + 1) * _VW],
                    p[:, :],
                    start=(t == 0), stop=(t == _NT - 1))

            def enqueue_finishers(qi):
                o, jq = qps[qi]
                sq = jq * 512
                av0, av1 = avs[qi]
                for hh, av in ((2 * o, av0), (2 * o + 1, av1)):
                    bp = 64 * (hh % 2)
                    invZ = pr.tile([1, 512], f32r, tag="invz", name="invZ")
                    bc_sb = pr.tile([64, 512], f32, tag="bc_sb", name="bc_sb")

                    def recip(av=av, invZ=invZ):
                        nc.vector.reciprocal(invZ[:], av[D_HEAD:_VW, :])

                    def bcast(invZ=invZ, bc_sb=bc_sb):
                        # PE ones-matmul broadcast via a borrowed sc slot
                        bc = ps_sc.tile([128, 1024], f32, tag="sc", name="bc")
                        nc.tensor.matmul(bc[0:64, :512], ones_col[:],
                                         invZ[:], start=True, stop=True)
                        nc.vector.tensor_copy(bc_sb[:], bc[0:64, :512])

                    def mult(bp=bp, av=av, bc_sb=bc_sb, o=o, sq=sq):
                        nc.vector.tensor_tensor(
                            attnT[bp:bp + 64, o, sq:sq + 512],
                            av[0:D_HEAD, :], bc_sb[:], ALU.mult)
                    jobs.append(recip)
                    jobs.append(bcast)
                    jobs.append(mult)

            def out_sg(sg):
                po = ps_sc.tile([128, 1024], f32, tag="sc", name="po")
                for o in range(DQ // 128):
                    nc.tensor.matmul(
                        po[:, :512],
                        attnT[:, o, sg * 128:(sg + 1) * 128],
                        w_sb["wh"][:, o, :],
                        start=(o == 0), stop=(o == DQ // 128 - 1))
                ot = pr.tile([128, 512], f32, tag="ot")
                nc.vector.tensor_copy(ot[:], po[:, :512])
                oeng = nc.sync if sg % 2 == 0 else nc.gpsimd
                oeng.dma_start(
                    o_d.rearrange("(t p) d -> p t d", p=128)[:, sg, :], ot[:])

            for o in (0, 1):
                for sg in range(4):
                    proj_group(kT, "wk", "bk", o, sg)
            proj_group(qT, "wq", "bq", 0, 0)
            proj_group(qT, "wq", "bq", 1, 0)
            project_v(0, 4)
            for sg in (1, 2, 3):
                proj_group(qT, "wq", "bq", 0, sg)
                proj_group(qT, "wq", "bq", 1, sg)
            project_v(4, 16)

            # h0's av rides 1 step behind its scores (ScalarE exp fits in
            # one t-period); h1's av rides 2 steps behind so DVE's linexp
            # plus one drained job still lands before the matmul needs it.
            pend0 = None
            pend1 = []
            for s in range(8 * _NT + 2):
                probs = None
                if s < 8 * _NT:
                    qi, t = divmod(s, _NT)
                    o, jq = qps[qi]
                    sq = jq * 512
                    if t == 0:
                        av0 = ps_av.tile([128, 512], f32, tag="av", name="av0")
                        av1 = ps_av.tile([128, 512], f32, tag="av", name="av1")
                        avs[qi] = (av0, av1)
                    sc = ps_sc.tile([128, 1024], f32, tag="sc", name="sc")
                    nc.tensor.matmul(
                        sc[:, 0:512],
                        kT[0:64, o, t * 128:(t + 1) * 128],
                        qT[0:64, o, sq:sq + 512],
                        start=True, stop=True)
                    nc.tensor.matmul(
                        sc[:, 512:1024],
                        kT[64:128, o, t * 128:(t + 1) * 128],
                        qT[64:128, o, sq:sq + 512],
                        start=True, stop=True)
                    # exp split per head-half: h0 exact on ScalarE; h1 on
                    # DVE linexp except every 4th t (error budget).  Each
                    # [128,512] half finishes within its pipeline lag, so
                    # the software pipeline rarely stalls on exp.
                    pf0 = pr.tile([128, 512], f16, tag="probs", name="pf0")
                    nc.scalar.activation(pf0[:], sc[:, 0:512], AF.Exp,
                                         scale=float(SCALE))
                    if t % 4 == 3:
                        pf1 = pr.tile([128, 512], f16, tag="probs", name="pf1")
                        nc.scalar.activation(pf1[:], sc[:, 512:1024], AF.Exp,
                                             scale=float(SCALE))
                        p1 = pf1
                    else:
                        pi1 = pr.tile([128, 512], i16, tag="probs", name="pi1")
                        nc.vector.tensor_scalar(
                            pi1[:], sc[:, 512:1024], _C1, _C2,
                            ALU.mult, ALU.add)
                        p1 = pi1.bitcast(f16)
                    probs = (pf0, p1)
                if len(pend1) == 2 or (s >= 8 * _NT and pend1):
                    qi1, t1, p1d = pend1.pop(0)
                    emit_av_h(qi1, t1, p1d, 1)
                    if t1 == _NT - 1:
                        enqueue_finishers(qi1)
                if pend0 is not None:
                    emit_av_h(pend0[0], pend0[1], pend0[2], 0)
                if s < 8 * _NT:
                    pend0 = (qi, t, probs[0])
                    pend1.append((qi, t, probs[1]))
                else:
                    pend0 = None
                if s < 8 * _NT and t % 2 == 1 and jobs:
                    jobs.pop(0)()
            while jobs:
                jobs.pop(0)()
            for sg in range(16):
                out_sg(sg)

    _split_excess_waits(nc)
    return nc


def _in_maps(inputs):
    x = np.ascontiguousarray(np.asarray(inputs["x"], dtype=np.float32))
    maps = []
    for c in range(N_CORES):
        b, g = c // 2, c % 2
        hs = slice(g * DQ, (g + 1) * DQ)
        xT = np.ascontiguousarray(x[b].T)                      # [512, 2048]
        maps.append({
            "x": xT,
            "wq": np.ascontiguousarray(np.asarray(inputs["Wq"], np.float32)[:, hs]),
            "wk": np.ascontiguousarray(np.asarray(inputs["Wk"], np.float32)[:, hs]),
            "wv": np.ascontiguousarray(np.asarray(inputs["Wv"], np.float32)[:, hs]),
            "wh": np.ascontiguousarray(np.asarray(inputs["Wh"], np.float32)[hs, :]),
            "bq": np.ascontiguousarray(np.asarray(inputs["bq"], np.float32)[hs]),
            "bk": np.ascontiguousarray(np.asarray(inputs["bk"], np.float32)[hs]),
        })
    return maps


def kernel(**inputs):
    from concourse.bass_utils import run_bass_kernel_spmd

    nc = build_nc(nrep=1)
    maps = _in_maps(inputs)
    res = run_bass_kernel_spmd(nc, maps, core_ids=list(range(N_CORES)))
    bh = np.asarray(inputs["bh"], np.float32)
    bv = np.asarray(inputs["bv"], np.float32)
    wh = np.asarray(inputs["Wh"], np.float32)
    bh_eff = bh + bv @ wh
    out = np.empty((B, S, D_MODEL), np.float32)
    for b in range(B):
        out[b] = res.results[2 * b]["o"] + res.results[2 * b + 1]["o"] + bh_eff
    return out


# revision 34
# speedup vs baseline: 20.0673x; 20.0673x over previous
"""Multi-head self-attention (B=4, S=2048, D=512, H=8, d=64) on 8 trn2 cores.

Sharding: 2 cores per batch element; each core computes 4 heads (a 256-wide
column slice of Wq/Wk/Wv and row slice of Wh) and produces a partial
[S, 512] output; the host sums the two partials per batch and adds
bh_eff = bh + bv @ Wh (the V-bias folded out of the device loop).

Per-core pipeline (measured ~153.5us/rep on HW, PE-work floor 139.9us):
  A) f32r projections of qT/kT (bias via ScalarE Identity) and of v,
     which is stored fp16, augmented with a ones column per head so the
     attention matmul also produces the softmax denominator row.
     (fp8 DoubleRow projections give 2x on these K=512 matmuls but the
     ~2% q/k element noise passes through softmax UNaveraged — attn is
     itself a random-sign average over values — costing 4e-2 rel err.)
  B) attention as ONE continuous 128-step software-pipelined stream over
     all 8 (head-pair, sq-quarter) windows; per step: the sc pair for
     step s (heads packed on PE row-groups, K=64 at base partitions
     0/64), exp of s, then the av matmuls — h0 rides 1 step behind its
     scores (ScalarE true Exp -> fp16, 612ns fits the 854ns t-period),
     h1 rides 2 steps behind (DVE linear-exp: i16(x*c1+c2) bitcast to
     fp16, a 2^x segment approximation, bias-centred, 3 of every 4 t;
     the extra step absorbs DVE's queued finisher jobs).  Window
     finishers (Z-row reciprocal, 1/Z ones-matmul broadcast via a
     borrowed sc slot, and the normalize multiplies into attnT) drain
     one-per-odd-t from a job queue, so no engine sees a burst.
  C) out[s,:] = attnT.T @ Wh as a post-stream phase; with qT/kT/v_aug
     double-buffered by rep parity it overlaps the next rep's
     projections, which is what the nrep-slope timing measures.

PSUM: 2x sc [128,1024] (4 banks, also lent to bc-broadcast and out-proj
tiles) + 4x av [128,512] (avs live across one window boundary for the
deferred normalize).  PE is the wall: scores 131k + attnV 131k cycles
are shape/port-bound at 128 elem/cycle (fp8 cannot help K<=128
matmuls), projections 49k, out 16k, bc 8k.  The exp split keeps ScalarE
(~98us) and DVE (~96us) under it.
"""

import numpy as np

NUM_HEADS = 8
D_MODEL = 512
D_HEAD = 64
B = 4
S = 2048
H_PER_CORE = 4          # heads per core
DQ = H_PER_CORE * D_HEAD  # 256 = per-core q/k/v width
N_CORES = 8
SCALE = 1.0 / np.sqrt(D_HEAD)

_KO = D_MODEL // 128    # 4 contraction chunks for the projections
_NT = S // 128          # 16 tiles of 128 along S
_VW = D_HEAD + 1        # 65: v columns per head incl. ones column

# DVE linear-exp: exp(SCALE*x) ~= bitcast_f16(i16(x*C1 + C2)).
# C2 includes -58.7 to centre the 2^frac-vs-1+frac sawtooth (mean bias
# e^0.0397), so DVE-half probs are not systematically ~4% above the
# ScalarE-half probs feeding the same softmax row.
_C1 = float(SCALE * np.log2(np.e) * 1024.0)
_C2 = float(15 * 1024 - 58.66)


def _split_excess_waits(nc):
    """Walrus's TRN2 codegen fits very few sync-waits per instruction (one on
    a Matmult's weight-load, few on drains).  Move excess waits onto NoOps
    inserted just before the instruction — engine queues are in-order, so a
    wait on a preceding same-engine instruction still protects it."""
    import concourse.mybir as mybir

    n_fixed = 0
    for f in nc.m.functions:
        for bb in f.blocks:
            insts = list(bb.instructions)
            out = []
            changed = False
            for ins in insts:
                si = ins.sync_info
                if si is not None and si.on_wait and len(si.on_wait) > 1:
                    waits = list(si.on_wait)
                    # An exp/matmul waiting on its OWN engine's completion sem
                    # is a slot-recycle WAW guard: implied by in-order issue,
                    # with the interleaved cross-engine reader guarded by the
                    # remaining wait.  Dropping it avoids a NoOp on the
                    # bottleneck queue (one per exp otherwise).
                    if isinstance(ins, (mybir.InstActivation, mybir.InstMatmult)):
                        eng_pfx = str(ins.engine).split(".")[-1] + "_"
                        cross = [w for w in waits
                                 if not str(getattr(w, "ant_name", "")).startswith(eng_pfx)]
                        if cross and len(cross) < len(waits):
                            waits = cross
                    for j, w in enumerate(waits[1:]):
                        nop = mybir.InstNoOp(
                            name=f"{ins.name}_waitnop{j}", ins=[], outs=[])
                        nop.engine = ins.engine
                        nop.sync_info = mybir.SyncInfo(on_wait=[w], on_update=[])
                        out.append(nop)
                    ins.sync_info = mybir.SyncInfo(
                        on_wait=waits[:1], on_update=list(si.on_update or []))
                    n_fixed += 1
                    changed = True
                out.append(ins)
            if changed:
                bb.instructions = out
    return n_fixed


def build_nc(nrep=1):
    """Build the per-core Bass program.  nrep>1 repeats the compute body
    (same tiles, idempotent) for wall-clock timing amplification."""
    import concourse.bass as bass
    import concourse.mybir as mybir
    import concourse.tile as tile

    f32 = mybir.dt.float32
    f32r = mybir.dt.float32r
    f16 = mybir.dt.float16
    i16 = mybir.dt.int16
    AF = mybir.ActivationFunctionType
    ALU = mybir.AluOpType

    nc = bass.Bass()
    x_d = nc.dram_tensor("x", [D_MODEL, S], f32r, kind="ExternalInput")
    wq_d = nc.dram_tensor("wq", [D_MODEL, DQ], f32r, kind="ExternalInput")
    wk_d = nc.dram_tensor("wk", [D_MODEL, DQ], f32r, kind="ExternalInput")
    wv_d = nc.dram_tensor("wv", [D_MODEL, DQ], f32r, kind="ExternalInput")
    wh_d = nc.dram_tensor("wh", [DQ, D_MODEL], f32r, kind="ExternalInput")
    bq_d = nc.dram_tensor("bq", [DQ], f32, kind="ExternalInput")
    bk_d = nc.dram_tensor("bk", [DQ], f32, kind="ExternalInput")
    o_d = nc.dram_tensor("o", [S, D_MODEL], f32, kind="ExternalOutput")

    with (
        nc.allow_low_precision(reason="f32r/fp16/fp8 attention pipeline"),
        tile.TileContext(nc) as tc,
        tc.tile_pool(name="cst", bufs=1) as cst,
        tc.tile_pool(name="big", bufs=1) as big,
        tc.tile_pool(name="pr", bufs=6) as pr,
        tc.tile_pool(name="ps_sc", bufs=2, space="PSUM") as ps_sc,
        tc.tile_pool(name="ps_av", bufs=4, space="PSUM") as ps_av,
    ):
        ones_col = cst.tile([1, D_HEAD], f32r)
        nc.gpsimd.memset(ones_col.bitcast(f32)[:], 1.0)

        # ---- phased input DMAs: what the first projections need gets the
        # full DMA-engine pool; the rest is gated behind a 1-elem copy that
        # depends on the first x chunk, so it cannot steal early bandwidth.
        w_shp = {"wq": (128, _KO, DQ), "wk": (128, _KO, DQ),
                 "wv": (128, _KO, DQ), "wh": (128, DQ // 128, D_MODEL)}
        w_dram = {"wq": wq_d, "wk": wk_d, "wv": wv_d, "wh": wh_d}
        w_sb = {}
        for n, s in w_shp.items():
            wt = big.tile(list(s), f32r, tag=f"w_{n}", name=f"w_{n}")
            w_sb[n] = wt
        xT = big.tile([128, _KO, S], f32r, tag="xT")
        x_rr = x_d.rearrange("(a p) s -> p a s", p=128)

        def load_x(sg, eng):
            eng.dma_start(xT[:, :, sg * 256:(sg + 1) * 256],
                          x_rr[:, :, sg * 256:(sg + 1) * 256])

        nc.sync.dma_start(w_sb["wk"][:],
                          w_dram["wk"].rearrange("(a p) m -> p a m", p=128))
        for sg, eng in ((0, nc.scalar), (1, nc.gpsimd), (2, nc.sync),
                        (3, nc.scalar)):
            load_x(sg, eng)
        bias_sb = {}
        for name, dram in (("bq", bq_d), ("bk", bk_d)):
            bt = cst.tile([128, DQ // 128], f32, tag=f"b_{name}")
            nc.gpsimd.dma_start(bt[:], dram.rearrange("(o p) -> p o", p=128))
            bias_sb[name] = bt
        # gate: a dependent 1-elem copy; phase-2 DMAs queue behind it
        gate = cst.tile([1, 1], f32)
        nc.gpsimd.tensor_copy(gate[:], xT.bitcast(f32)[0:1, 0, 0:1])
        for sg, eng in ((4, nc.gpsimd), (5, nc.gpsimd), (6, nc.gpsimd),
                        (7, nc.gpsimd)):
            load_x(sg, eng)
        for name in ("wq", "wv", "wh"):
            nc.gpsimd.dma_start(
                w_sb[name][:],
                w_dram[name].rearrange("(a p) m -> p a m", p=128))

        for _rep in range(nrep):
            # qT/kT/v_aug alternate buffers by rep parity so rep k+1's
            # projections never WAR-serialize against rep k's attention
            # (the nrep>1 timing build measures steady-state throughput;
            # with one buffer the proj/attention overlap is lost at every
            # rep boundary).  attnT needs no alternation: its first write
            # in rep k+1 happens ~a full window after rep k's last read.
            par = _rep % 2
            qT = big.tile([128, DQ // 128, S], f32r, tag=f"qT{par}",
                          name=f"qT{par}")
            kT = big.tile([128, DQ // 128, S], f32r, tag=f"kT{par}",
                          name=f"kT{par}")
            attnT = big.tile([128, DQ // 128, S], f32r, tag="attnT")
            v_aug = big.tile([128, _NT, H_PER_CORE * _VW], f16,
                             tag=f"v_aug{par}", name=f"v_aug{par}")
            nc.gpsimd.memset(v_aug[:], 1.0)

            def proj_group(dst, wname, bname, o, sg):
                p = ps_sc.tile([128, 1024], f32, tag="sc", name="p_qk")
                for ko in range(_KO):
                    nc.tensor.matmul(
                        p[:, :512],
                        w_sb[wname][:, ko, o * 128:(o + 1) * 128],
                        xT[:, ko, sg * 512:(sg + 1) * 512],
                        start=(ko == 0), stop=(ko == _KO - 1))
                nc.scalar.activation(
                    dst[:, o, sg * 512:(sg + 1) * 512], p[:, :512],
                    AF.Identity, bias=bias_sb[bname][:, o:o + 1])

            def project_v(t0, t1):
                for t in range(t0, t1):
                    pv = ps_av.tile([128, 512], f32, tag="av", name="pv")
                    for ko in range(_KO):
                        nc.tensor.matmul(
                            pv[:, :DQ],
                            xT[:, ko, t * 128:(t + 1) * 128],
                            w_sb["wv"][:, ko, :],
                            start=(ko == 0), stop=(ko == _KO - 1))
                    nc.vector.tensor_copy(
                        v_aug[:, t, :].rearrange(
                            "p (h w) -> p h w", w=_VW)[:, :, :D_HEAD],
                        pv[:, :DQ].rearrange("p (h w) -> p h w", w=D_HEAD))

            # ---- attention: one continuous 128-step stream over all 8
            # (pair, quarter) windows.  Per step: sc-pair for step s, exp(s),
            # av-pair for step s-1 — so each window's first scores fill the
            # previous window's exp-latency tail instead of stalling behind
            # its last av.  Window finishers (reciprocal+broadcast, the
            # normalize multiplies, and the out-projection sg-pieces) go into
            # a job queue drained one-per-odd-t, spreading DVE/PE extras
            # evenly between the exp slots.
            qps = [(o, jq) for jq in range(4) for o in (0, 1)]
            avs = {}
            jobs = []

            def emit_av_h(qi, t, p, which):
                o, jq = qps[qi]
                av = avs[qi][which]
                hh = 2 * o + which
                nc.tensor.matmul(
                    av[0:_VW, :],
                    v_aug[:, t, hh * _VW:(hh + 1) * _VW],
                    p[:, :],
                    start=(t == 0), stop=(t == _NT - 1))

            def enqueue_finishers(qi):
                o, jq = qps[qi]
                sq = jq * 512
                av0, av1 = avs[qi]
                for hh, av in ((2 * o, av0), (2 * o + 1, av1)):
                    bp = 64 * (hh % 2)
                    invZ = pr.tile([1, 512], f32r, tag="invz", name="invZ")
                    bc_sb = pr.tile([64, 512], f32, tag="bc_sb", name="bc_sb")

                    def recip(av=av, invZ=invZ):
                        nc.vector.reciprocal(invZ[:], av[D_HEAD:_VW, :])

                    def bcast(invZ=invZ, bc_sb=bc_sb):
                        # PE ones-matmul broadcast via a borrowed sc slot
                        bc = ps_sc.tile([128, 1024], f32, tag="sc", name="bc")
                        nc.tensor.matmul(bc[0:64, :512], ones_col[:],
                                         invZ[:], start=True, stop=True)
                        nc.vector.tensor_copy(bc_sb[:], bc[0:64, :512])

                    def mult(bp=bp, av=av, bc_sb=bc_sb, o=o, sq=sq):
                        nc.vector.tensor_tensor(
                            attnT[bp:bp + 64, o, sq:sq + 512],
                            av[0:D_HEAD, :], bc_sb[:], ALU.mult)
                    jobs.append(recip)
                    jobs.append(bcast)
                    jobs.append(mult)

            def out_sg(sg):
                po = ps_sc.tile([128, 1024], f32, tag="sc", name="po")
                for o in range(DQ // 128):
                    nc.tensor.matmul(
                        po[:, :512],
                        attnT[:, o, sg * 128:(sg + 1) * 128],
                        w_sb["wh"][:, o, :],
                        start=(o == 0), stop=(o == DQ // 128 - 1))
                ot = pr.tile([128, 512], f32, tag="ot")
                nc.vector.tensor_copy(ot[:], po[:, :512])
                oeng = nc.sync if sg % 2 == 0 else nc.gpsimd
                oeng.dma_start(
                    o_d.rearrange("(t p) d -> p t d", p=128)[:, sg, :], ot[:])

            for o in (0, 1):
                for sg in range(4):
                    proj_group(kT, "wk", "bk", o, sg)
            proj_group(qT, "wq", "bq", 0, 0)
            proj_group(qT, "wq", "bq", 1, 0)
            project_v(0, 4)
            for sg in (1, 2, 3):
                proj_group(qT, "wq", "bq", 0, sg)
                proj_group(qT, "wq", "bq", 1, sg)
            project_v(4, 16)

            # h0's av rides 1 step behind its scores (ScalarE exp fits in
            # one t-period); h1's av rides 2 steps behind so DVE's linexp
            # plus one drained job still lands before the matmul needs it.
            pend0 = None
            pend1 = []
            for s in range(8 * _NT + 2):
                probs = None
                if s < 8 * _NT:
                    qi, t = divmod(s, _NT)
                    o, jq = qps[qi]
                    sq = jq * 512
                    if t == 0:
                        av0 = ps_av.tile([128, 512], f32, tag="av", name="av0")
                        av1 = ps_av.tile([128, 512], f32, tag="av", name="av1")
                        avs[qi] = (av0, av1)
                    sc = ps_sc.tile([128, 1024], f32, tag="sc", name="sc")
                    nc.tensor.matmul(
                        sc[:, 0:512],
                        kT[0:64, o, t * 128:(t + 1) * 128],
                        qT[0:64, o, sq:sq + 512],
                        start=True, stop=True)
                    nc.tensor.matmul(
                        sc[:, 512:1024],
                        kT[64:128, o, t * 128:(t + 1) * 128],
                        qT[64:128, o, sq:sq + 512],
                        start=True, stop=True)
                    # exp split per head-half: h0 exact on ScalarE; h1 on
                    # DVE linexp except every 4th t (error budget).  Each
                    # [128,512] half finishes within its pipeline lag, so
                    # the software pipeline rarely stalls on exp.
                    pf0 = pr.tile([128, 512], f16, tag="probs", name="pf0")
                    nc.scalar.activation(pf0[:], sc[:, 0:512], AF.Exp,
                                         scale=float(SCALE))
                    if t % 4 == 3:
                        pf1 = pr.tile([128, 512], f16, tag="probs", name="pf1")
                        nc.scalar.activation(pf1[:], sc[:, 512:1024], AF.Exp,
                                             scale=float(SCALE))
                        p1 = pf1
                    else:
                        pi1 = pr.tile([128, 512], i16, tag="probs", name="pi1")
                        nc.vector.tensor_scalar(
                            pi1[:], sc[:, 512:1024], _C1, _C2,
                            ALU.mult, ALU.add)
                        p1 = pi1.bitcast(f16)
                    probs = (pf0, p1)
                if len(pend1) == 2 or (s >= 8 * _NT and pend1):
                    qi1, t1, p1d = pend1.pop(0)
                    emit_av_h(qi1, t1, p1d, 1)
                    if t1 == _NT - 1:
                        enqueue_finishers(qi1)
                if pend0 is not None:
                    emit_av_h(pend0[0], pend0[1], pend0[2], 0)
                if s < 8 * _NT:
                    pend0 = (qi, t, probs[0])
                    pend1.append((qi, t, probs[1]))
                else:
                    pend0 = None
                if s < 8 * _NT and t % 2 == 1 and jobs:
                    jobs.pop(0)()
            while jobs:
                jobs.pop(0)()
            for sg in range(16):
                out_sg(sg)

    _split_excess_waits(nc)
    return nc


def _in_maps(inputs):
    x = np.ascontiguousarray(np.asarray(inputs["x"], dtype=np.float32))
    maps = []
    for c in range(N_CORES):
        b, g = c // 2, c % 2
        hs = slice(g * DQ, (g + 1) * DQ)
        xT = np.ascontiguousarray(x[b].T)                      # [512, 2048]
        maps.append({
            "x": xT,
            "wq": np.ascontiguousarray(np.asarray(inputs["Wq"], np.float32)[:, hs]),
            "wk": np.ascontiguousarray(np.asarray(inputs["Wk"], np.float32)[:, hs]),
            "wv": np.ascontiguousarray(np.asarray(inputs["Wv"], np.float32)[:, hs]),
            "wh": np.ascontiguousarray(np.asarray(inputs["Wh"], np.float32)[hs, :]),
            "bq": np.ascontiguousarray(np.asarray(inputs["bq"], np.float32)[hs]),
            "bk": np.ascontiguousarray(np.asarray(inputs["bk"], np.float32)[hs]),
        })
    return maps


def kernel(**inputs):
    from concourse.bass_utils import run_bass_kernel_spmd

    nc = build_nc(nrep=1)
    maps = _in_maps(inputs)
    res = run_bass_kernel_spmd(nc, maps, core_ids=list(range(N_CORES)))
    bh = np.asarray(inputs["bh"], np.float32)
    bv = np.asarray(inputs["bv"], np.float32)
    wh = np.asarray(inputs["Wh"], np.float32)
    bh_eff = bh + bv @ wh
    out = np.empty((B, S, D_MODEL), np.float32)
    for b in range(B):
        out[b] = res.results[2 * b]["o"] + res.results[2 * b + 1]["o"] + bh_eff
    return out
